# revision 13
# baseline (speedup 1.0000x reference)
"""Trainium2 Bass kernel for nn_AttentionBlock (B=4, C=512, T=2048, H=8, G=32).

Sharding: 8 cores = (batch b in 0..3) x (head-group hg in 0..1, 4 heads each).
Each core computes groupnorm(x[b]) (redundantly within the pair), its heads'
q/k/v, attention, and a partial projection using its head-group's w_proj
columns.  Host sums the two partials per batch; the hg==0 core folds in the
residual x and b_proj.

On-chip layout highlights:
 - QK^T computed in scoresT[s, t] layout; two heads of a pair occupy
   partition halves 0-63 / 64-127 so their K=64 matmuls run concurrently on
   distinct PE row-groups.
 - exp via ACT psum->sbuf, no max subtraction (scores are O(5), safe in fp32).
 - V generated directly s-major (lhsT = h) and augmented with a ones column,
   so the PV matmul emits both out^T[d, t] and the softmax row sums.
 - softmax division: reciprocal of sums on DVE, broadcast across partitions
   via a K=1 ones matmul (exact fp32), then one DVE multiply.
 - big matmuls use fp32r (full-rate); tiny stat/broadcast matmuls use exact
   fp32.
"""

import sys
from contextlib import ExitStack

sys.path.insert(0, "/opt/trn_rl_repo")

import numpy as np

import concourse.bass as bass
import concourse.tile as tile
from concourse import bacc, mybir
from concourse.bass_utils import run_bass_kernel_spmd

F32 = mybir.dt.float32
F32R = mybir.dt.float32r
AF = mybir.ActivationFunctionType
OP = mybir.AluOpType

B, C, T = 4, 512, 2048
H = 8
DH = C // H          # 64
G = 32               # groupnorm groups
GS = C // G          # 16 channels per group
EPS = 1e-5
NKC = C // 128       # 4 c-chunks
NTC4 = T // 512      # 4 t-chunks of 512
SCALE = 1.0 / np.sqrt(np.sqrt(DH))

_CACHE = {}


def round_f32r(a):
    u = np.ascontiguousarray(a, np.float32).view(np.uint32)
    low = u & np.uint32(0xFFF)
    base = u & ~np.uint32(0xFFF)
    lsb = (base >> np.uint32(12)) & np.uint32(1)
    up = (low > 0x800) | ((low == 0x800) & (lsb == 1))
    out = base + (up.astype(np.uint32) << np.uint32(12))
    return out.view(np.float32)


def r(ap):
    return ap.bitcast(F32R)


def build_program():
    nc = bacc.Bacc("TRN2", target_bir_lowering=False, debug=False)

    def inp(name, shape, dt=F32):
        return nc.dram_tensor(name, shape, dt, kind="ExternalInput").ap()

    x_d = inp("x", [C, T])
    wqk_d = inp("wqk", [C, 512], F32R)
    bqk_d = inp("bqk", [128, 4])
    wv_d = inp("wv", [C, 260], F32R)
    bv_d = inp("bv", [1, 260], F32R)
    wp_d = inp("wp", [256, 512], F32R)
    bp_d = inp("bp", [128, 4])
    rs_d = inp("rs", [128, 1])
    gamma_d = inp("gamma", [128, 4])
    beta_d = inp("beta", [128, 4])
    ones16_d = inp("ones16", [128, 8])
    expander_d = inp("expander", [8, 128])
    onest_d = inp("onest", [1, T], F32R)
    sel_d = inp("sel", [8, 512])
    y_d = nc.dram_tensor("y", [C, T], F32, kind="ExternalOutput").ap()

    x_r = x_d.rearrange("(k p) t -> k p t", p=128)
    wqk_r = wqk_d.rearrange("(k p) m -> k p m", p=128)
    wv_r = wv_d.rearrange("(k p) m -> k p m", p=128)
    wp_r = wp_d.rearrange("(k p) m -> k p m", p=128)
    y_r = y_d.rearrange("(k p) t -> k p t", p=128)

    with tile.TileContext(nc) as tc, ExitStack() as ctx:
        consts = ctx.enter_context(tc.tile_pool(name="consts", bufs=1))
        xpool = ctx.enter_context(tc.tile_pool(name="xpool", bufs=4))
        big = ctx.enter_context(tc.tile_pool(name="big", bufs=4))
        qkpool = ctx.enter_context(tc.tile_pool(name="qkpool", bufs=4))
        vtpool = ctx.enter_context(tc.tile_pool(name="vtpool", bufs=16))
        ptpool = ctx.enter_context(tc.tile_pool(name="ptpool", bufs=4))
        ysb = ctx.enter_context(tc.tile_pool(name="ysb", bufs=6))
        small = ctx.enter_context(tc.tile_pool(name="small", bufs=10))
        small2 = ctx.enter_context(tc.tile_pool(name="small2", bufs=1))
        pp_sc = ctx.enter_context(tc.tile_pool(name="pp_sc", bufs=3, space="PSUM"))
        pp_out = ctx.enter_context(tc.tile_pool(name="pp_out", bufs=1, space="PSUM"))

        ctr = [0]

        def psum_sc():
            ctr[0] += 1
            return pp_sc.tile([128, 1024], F32, tag="sc", name=f"sc{ctr[0]}")

        def psum_out():
            ctr[0] += 1
            return pp_out.tile([128, 1024], F32, tag="po", name=f"po{ctr[0]}")

        # ---- load constants ----
        wqk_sb = consts.tile([128, NKC, 512], F32R)
        wv_sb = consts.tile([128, NKC, 260], F32R)
        wp_sb = consts.tile([128, 2, 512], F32R)
        for kc in range(NKC):
            nc.sync.dma_start(out=wqk_sb[:, kc, :], in_=wqk_r[kc])
            nc.sync.dma_start(out=wv_sb[:, kc, :], in_=wv_r[kc])
        for kc in range(2):
            nc.sync.dma_start(out=wp_sb[:, kc, :], in_=wp_r[kc])
        bqk_sb = consts.tile([128, 4], F32)
        nc.sync.dma_start(out=bqk_sb, in_=bqk_d)
        bv_sb = consts.tile([1, 260], F32R)
        nc.sync.dma_start(out=bv_sb, in_=bv_d)
        bp_sb = consts.tile([128, 4], F32)
        nc.sync.dma_start(out=bp_sb, in_=bp_d)
        rs_sb = consts.tile([128, 1], F32)
        nc.sync.dma_start(out=rs_sb, in_=rs_d)
        gamma_sb = consts.tile([128, 4], F32)
        nc.sync.dma_start(out=gamma_sb, in_=gamma_d)
        beta_sb = consts.tile([128, 4], F32)
        nc.sync.dma_start(out=beta_sb, in_=beta_d)
        ones16_sb = consts.tile([128, 8], F32)
        nc.sync.dma_start(out=ones16_sb, in_=ones16_d)
        expander_sb = consts.tile([8, 128], F32)
        nc.sync.dma_start(out=expander_sb, in_=expander_d)
        onest_sb = consts.tile([1, T], F32R)
        nc.sync.dma_start(out=onest_sb, in_=onest_d)
        eps_sb = consts.tile([128, 1], F32)
        nc.vector.memset(eps_sb, EPS)
        sel_sb = consts.tile([8, 512], F32)
        nc.sync.dma_start(out=sel_sb, in_=sel_d)

        # ---- groupnorm ----
        xs = []
        hs = []
        for kc in range(NKC):
            x_t = xpool.tile([128, T], F32, tag="x")
            for j in range(4):
                nc.sync.dma_start(
                    out=x_t[:, j * 512 : (j + 1) * 512],
                    in_=x_r[kc][:, j * 512 : (j + 1) * 512],
                )
            xs.append(x_t)

            stats = small.tile([128, 4, 6], F32, tag="stats")
            for j in range(4):
                nc.vector.bn_stats(
                    out=stats[:, j, :], in_=x_t[:, j * 512 : (j + 1) * 512]
                )
            mv = small.tile([128, 2], F32, tag="mv")
            nc.vector.bn_aggr(out=mv, in_=stats)

            # pack rhs: col0 = mean_c, col1 = E[x^2]_c = var_c + mean_c^2
            pk = small.tile([128, 2], F32, tag="pk")
            nc.vector.tensor_copy(pk[:, 0:1], mv[:, 0:1])
            nc.vector.tensor_mul(pk[:, 1:2], mv[:, 0:1], mv[:, 0:1])
            nc.vector.tensor_add(pk[:, 1:2], pk[:, 1:2], mv[:, 1:2])

            ps_g = psum_out()
            nc.tensor.matmul(
                ps_g[0:8, 0:2], ones16_sb, pk, start=True, stop=True
            )
            # gm: col0 = mean_g, col1 = rstd_g
            gsum = small.tile([8, 2], F32, tag="gsum")
            nc.vector.tensor_copy(gsum, ps_g[0:8, 0:2])
            gm = small.tile([8, 2], F32, tag="gm")
            nc.vector.tensor_copy(gm[:, 0:1], gsum[:, 0:1])
            varg = small.tile([8, 1], F32, tag="varg")
            nc.vector.tensor_mul(varg, gsum[:, 0:1], gsum[:, 0:1])
            nc.vector.tensor_sub(varg, gsum[:, 1:2], varg)
            nc.scalar.activation(varg, varg, AF.Sqrt, bias=eps_sb[0:8, :])
            nc.vector.reciprocal(gm[:, 1:2], varg)

            ps_pc = psum_out()
            nc.tensor.matmul(
                ps_pc[0:128, 0:2], expander_sb, gm, start=True, stop=True
            )
            scale = small.tile([128, 1], F32, tag="scale")
            nc.vector.tensor_mul(scale, ps_pc[:, 1:2], gamma_sb[:, kc : kc + 1])
            nbias = small.tile([128, 1], F32, tag="nbias")
            nc.vector.tensor_mul(nbias, ps_pc[:, 0:1], scale)
            nc.vector.tensor_sub(nbias, beta_sb[:, kc : kc + 1], nbias)

            h_t = big.tile([128, T], F32, tag="big")
            nc.vector.tensor_scalar(
                out=r(h_t),
                in0=x_t,
                scalar1=scale,
                scalar2=nbias,
                op0=OP.mult,
                op1=OP.add,
            )
            hs.append(h_t)

        # ---- q/k generation: m-chunks [qP0, kP0, qP1, kP1] ----
        qk_tiles = []
        for mc in range(4):
            dest = qkpool.tile([128, T], F32, tag="qk")
            qk_tiles.append(dest)
            for tc2 in range(2):
                ps = psum_sc()
                for half in range(2):
                    t0 = (tc2 * 2 + half) * 512
                    for kc in range(NKC):
                        nc.tensor.matmul(
                            ps[:, half * 512 : half * 512 + 512],
                            r(wqk_sb[:, kc, mc * 128 : mc * 128 + 128]),
                            r(hs[kc][:, t0 : t0 + 512]),
                            start=(kc == 0),
                            stop=(kc == NKC - 1),
                        )
                nc.vector.tensor_scalar(
                    out=r(dest[:, tc2 * 1024 : tc2 * 1024 + 1024]),
                    in0=ps,
                    scalar1=bqk_sb[:, mc : mc + 1],
                    scalar2=None,
                    op0=OP.add,
                )
        qpair = [qk_tiles[0], qk_tiles[2]]
        kpair = [qk_tiles[1], qk_tiles[3]]

        # ---- v generation, s-major with ones column ----
        vts = []
        for sc in range(16):
            ps = psum_sc()
            for kc in range(NKC):
                nc.tensor.matmul(
                    ps[:, 0:260],
                    r(hs[kc][:, sc * 128 : sc * 128 + 128]),
                    r(wv_sb[:, kc, :]),
                    start=(kc == 0),
                    stop=False,
                )
            nc.tensor.matmul(
                ps[:, 0:260],
                r(onest_sb[0:1, sc * 128 : sc * 128 + 128]),
                r(bv_sb),
                start=False,
                stop=True,
            )
            vt = vtpool.tile([128, 4, 65], F32, tag="vt")
            nc.vector.tensor_copy(r(vt), ps[:, 0:260])
            vts.append(vt)

        # ---- attention (one head at a time; deep scores pipeline) ----
        sums_stage = small2.tile([8, 1024], F32, tag="sums")
        att = [big.tile([128, T], F32, tag="big", name=f"att{i}") for i in range(2)]
        for pr in range(2):
            qp, kp = qpair[pr], kpair[pr]
            for hip in range(2):
                hb = hip * 64
                for th in range(2):
                    u = (pr * 2 + hip) * 2 + th
                    outp = psum_out()
                    for sc in range(16):
                        ps = psum_sc()
                        for half in range(2):
                            t0 = th * 1024 + half * 512
                            f0 = half * 512
                            nc.tensor.matmul(
                                ps[:, f0 : f0 + 512],
                                r(kp[hb : hb + 64, sc * 128 : sc * 128 + 128]),
                                r(qp[hb : hb + 64, t0 : t0 + 512]),
                                start=True,
                                stop=True,
                            )
                        pt_t = ptpool.tile([128, 1024], F32, tag="pt")
                        nc.scalar.activation(r(pt_t), ps, AF.Exp)
                        vh = vts[sc][:, pr * 2 + hip, :]
                        for half in range(2):
                            f0 = half * 512
                            nc.tensor.matmul(
                                outp[0:65, f0 : f0 + 512],
                                r(vh),
                                r(pt_t[:, f0 : f0 + 512]),
                                start=(sc == 0),
                                stop=(sc == 15),
                            )
                    nc.vector.tensor_copy(
                        r(
                            att[pr][
                                hb : hb + 64,
                                th * 1024 : th * 1024 + 1024,
                            ]
                        ),
                        outp[0:64, :],
                    )
                    stg = ptpool.tile(
                        [128, 1024], F32, tag="pt", name=f"stg{u}"
                    )
                    nc.vector.tensor_copy(stg[64:65, :], outp[64:65, :])
                    nc.sync.dma_start(
                        out=sums_stage[u : u + 1, :], in_=stg[64:65, :]
                    )

        # ---- softmax division + att assembly (d-major) ----
        rb = small2.tile([8, 1024], F32, tag="rb")
        scr = small2.tile([8, 1024], F32, tag="scr")
        nc.vector.reciprocal_approx_accurate(out=rb, in_=sums_stage, scratch=scr)
        for pr in range(2):
            for hip in range(2):
                for th in range(2):
                    u = (pr * 2 + hip) * 2 + th
                    bc = psum_out()
                    for half in range(2):
                        f0 = half * 512
                        nc.tensor.matmul(
                            bc[0:64, f0 : f0 + 512],
                            sel_sb[:, u * 64 : u * 64 + 64],
                            rb[:, f0 : f0 + 512],
                            start=True,
                            stop=True,
                        )
                    a_slc = att[pr][
                        hip * 64 : hip * 64 + 64, th * 1024 : th * 1024 + 1024
                    ]
                    nc.vector.tensor_mul(r(a_slc), r(a_slc), bc[0:64, :])

        # ---- projection + residual ----
        for mc in range(4):
            for tc4 in range(NTC4):
                ps = psum_sc()
                for kc2 in range(2):
                    nc.tensor.matmul(
                        ps[:, 0:512],
                        r(wp_sb[:, kc2, mc * 128 : mc * 128 + 128]),
                        r(att[kc2][:, tc4 * 512 : tc4 * 512 + 512]),
                        start=(kc2 == 0),
                        stop=(kc2 == 1),
                    )
                xz = ysb.tile([128, 512], F32, tag="y")
                nc.vector.tensor_scalar(
                    out=xz,
                    in0=xs[mc][:, tc4 * 512 : tc4 * 512 + 512],
                    scalar1=rs_sb,
                    scalar2=bp_sb[:, mc : mc + 1],
                    op0=OP.mult,
                    op1=OP.add,
                )
                yt = ysb.tile([128, 512], F32, tag="y")
                nc.vector.tensor_add(yt, ps[:, 0:512], xz)
                nc.sync.dma_start(
                    out=y_r[mc][:, tc4 * 512 : tc4 * 512 + 512], in_=yt
                )

    nc.compile()
    return nc


def _consts():
    ones16 = np.zeros((128, 8), np.float32)
    for g in range(8):
        ones16[g * 16 : (g + 1) * 16, g] = 1.0 / GS
    expander = np.zeros((8, 128), np.float32)
    for g in range(8):
        expander[g, g * 16 : (g + 1) * 16] = 1.0
    onest = np.ones((1, T), np.float32)
    sel = np.zeros((8, 512), np.float32)
    for u in range(8):
        sel[u, u * 64 : (u + 1) * 64] = 1.0
    return ones16, expander, onest, sel


def _core_weights(hg, w_qkv, b_qkv, w_proj, b_proj, gn_gamma, gn_beta):
    heads = [4 * hg + i for i in range(4)]
    qrows, krows, vrows = [], [], []
    for h in heads:
        base = h * 3 * DH
        qrows.append(np.arange(base, base + DH))
        krows.append(np.arange(base + DH, base + 2 * DH))
        vrows.append(np.arange(base + 2 * DH, base + 3 * DH))
    # m-chunks: [qP0, kP0, qP1, kP1]; each pair chunk = [head_even | head_odd]
    qk_order = np.concatenate(
        [qrows[0], qrows[1], krows[0], krows[1], qrows[2], qrows[3], krows[2], krows[3]]
    )
    wqk = round_f32r(w_qkv[qk_order].T * SCALE)
    bqk = np.ascontiguousarray((b_qkv[qk_order] * SCALE).reshape(4, 128).T)
    wv = np.zeros((C, 260), np.float32)
    bv = np.zeros((1, 260), np.float32)
    for i, vr in enumerate(vrows):
        wv[:, i * 65 : i * 65 + 64] = w_qkv[vr].T
        bv[0, i * 65 : i * 65 + 64] = b_qkv[vr]
        bv[0, i * 65 + 64] = 1.0
    wv = round_f32r(wv)
    bv = round_f32r(bv)
    att_cols = np.concatenate([np.arange(h * DH, (h + 1) * DH) for h in heads])
    wp = round_f32r(w_proj[:, att_cols].T)
    if hg == 0:
        bp = np.ascontiguousarray(b_proj.reshape(4, 128).T)
        rs = np.ones((128, 1), np.float32)
    else:
        bp = np.zeros((128, 4), np.float32)
        rs = np.zeros((128, 1), np.float32)
    gamma = np.ascontiguousarray(gn_gamma.reshape(4, 128).T)
    beta = np.ascontiguousarray(gn_beta.reshape(4, 128).T)
    return dict(
        wqk=wqk, bqk=bqk, wv=wv, bv=bv, wp=wp, bp=bp, rs=rs,
        gamma=gamma, beta=beta,
    )


def kernel(x, gn_gamma, gn_beta, w_qkv, b_qkv, w_proj, b_proj, _trace=False):
    x = np.asarray(x, np.float32)
    gn_gamma = np.asarray(gn_gamma, np.float32)
    gn_beta = np.asarray(gn_beta, np.float32)
    w_qkv = np.asarray(w_qkv, np.float32)
    b_qkv = np.asarray(b_qkv, np.float32)
    w_proj = np.asarray(w_proj, np.float32)
    b_proj = np.asarray(b_proj, np.float32)

    if "nc" not in _CACHE:
        _CACHE["nc"] = build_program()
    nc = _CACHE["nc"]

    ones16, expander, onest, sel = _consts()
    hg_consts = [
        _core_weights(hg, w_qkv, b_qkv, w_proj, b_proj, gn_gamma, gn_beta)
        for hg in range(2)
    ]
    in_maps = []
    for core in range(8):
        b, hg = core // 2, core % 2
        m = dict(hg_consts[hg])
        m["x"] = np.ascontiguousarray(x[b])
        m["ones16"] = ones16
        m["expander"] = expander
        m["onest"] = onest
        m["sel"] = sel
        in_maps.append(m)

    res = run_bass_kernel_spmd(
        nc, in_maps, core_ids=list(range(8)), trace=_trace
    )
    y = np.empty((B, C, T), np.float32)
    for b in range(B):
        y[b] = res.results[2 * b]["y"] + res.results[2 * b + 1]["y"]
    if _trace:
        _CACHE["last_results"] = res
    return y


# revision 14
# speedup vs baseline: 1.6261x; 1.6261x over previous
"""Trainium2 Bass kernel for nn_AttentionBlock (B=4, C=512, T=2048, H=8, G=32).

Sharding: 8 cores = (batch b in 0..3) x (head-group hg in 0..1, 4 heads each).
Each core computes groupnorm(x[b]) (redundantly within the pair), its heads'
q/k/v, attention, and a partial projection using its head-group's w_proj
columns.  Host sums the two partials per batch; the hg==0 core folds in the
residual x and b_proj.

On-chip layout highlights:
 - QK^T computed in scoresT[s, t] layout; two heads of a pair occupy
   partition halves 0-63 / 64-127 so their K=64 matmuls run concurrently on
   distinct PE row-groups.
 - exp via ACT psum->sbuf, no max subtraction (scores are O(5), safe in fp32).
 - V generated directly s-major (lhsT = h) and augmented with a ones column,
   so the PV matmul emits both out^T[d, t] and the softmax row sums.
 - softmax division: reciprocal of sums on DVE, broadcast across partitions
   via a K=1 ones matmul (exact fp32), then one DVE multiply.
 - big matmuls use fp32r (full-rate); tiny stat/broadcast matmuls use exact
   fp32.
"""

import sys
from contextlib import ExitStack

sys.path.insert(0, "/opt/trn_rl_repo")

import numpy as np

import concourse.bass as bass
import concourse.tile as tile
from concourse import bacc, mybir
from concourse.bass_utils import run_bass_kernel_spmd

F32 = mybir.dt.float32
F32R = mybir.dt.float32r
AF = mybir.ActivationFunctionType
OP = mybir.AluOpType

B, C, T = 4, 512, 2048
H = 8
DH = C // H          # 64
G = 32               # groupnorm groups
GS = C // G          # 16 channels per group
EPS = 1e-5
NKC = C // 128       # 4 c-chunks
NTC4 = T // 512      # 4 t-chunks of 512
SCALE = 1.0 / np.sqrt(np.sqrt(DH))

_CACHE = {}


def round_f32r(a):
    u = np.ascontiguousarray(a, np.float32).view(np.uint32)
    low = u & np.uint32(0xFFF)
    base = u & ~np.uint32(0xFFF)
    lsb = (base >> np.uint32(12)) & np.uint32(1)
    up = (low > 0x800) | ((low == 0x800) & (lsb == 1))
    out = base + (up.astype(np.uint32) << np.uint32(12))
    return out.view(np.float32)


def r(ap):
    return ap.bitcast(F32R)


def build_program():
    nc = bacc.Bacc("TRN2", target_bir_lowering=False, debug=False)

    def inp(name, shape, dt=F32):
        return nc.dram_tensor(name, shape, dt, kind="ExternalInput").ap()

    x_d = inp("x", [C, T])
    wqk_d = inp("wqk", [C, 512], F32R)
    bqk_d = inp("bqk", [128, 4])
    wv_d = inp("wv", [C, 260], F32R)
    bv_d = inp("bv", [1, 260], F32R)
    wp_d = inp("wp", [256, 512], F32R)
    bp_d = inp("bp", [128, 4])
    rs_d = inp("rs", [128, 1])
    gamma_d = inp("gamma", [128, 4])
    beta_d = inp("beta", [128, 4])
    ones16_d = inp("ones16", [128, 8])
    expander_d = inp("expander", [8, 128])
    onest_d = inp("onest", [1, T], F32R)
    sel_d = inp("sel", [8, 512], F32R)
    y_d = nc.dram_tensor("y", [C, T], F32, kind="ExternalOutput").ap()

    x_r = x_d.rearrange("(k p) t -> k p t", p=128)
    wqk_r = wqk_d.rearrange("(k p) m -> k p m", p=128)
    wv_r = wv_d.rearrange("(k p) m -> k p m", p=128)
    wp_r = wp_d.rearrange("(k p) m -> k p m", p=128)
    y_r = y_d.rearrange("(k p) t -> k p t", p=128)

    with tile.TileContext(nc) as tc, ExitStack() as ctx:
        consts = ctx.enter_context(tc.tile_pool(name="consts", bufs=1))
        xpool = ctx.enter_context(tc.tile_pool(name="xpool", bufs=4))
        big = ctx.enter_context(tc.tile_pool(name="big", bufs=4))
        qkpool = ctx.enter_context(tc.tile_pool(name="qkpool", bufs=4))
        vtpool = ctx.enter_context(tc.tile_pool(name="vtpool", bufs=16))
        ptpool = ctx.enter_context(tc.tile_pool(name="ptpool", bufs=4))
        ysb = ctx.enter_context(tc.tile_pool(name="ysb", bufs=6))
        small = ctx.enter_context(tc.tile_pool(name="small", bufs=10))
        small2 = ctx.enter_context(tc.tile_pool(name="small2", bufs=1))
        pp_sc = ctx.enter_context(tc.tile_pool(name="pp_sc", bufs=3, space="PSUM"))
        pp_out = ctx.enter_context(tc.tile_pool(name="pp_out", bufs=2, space="PSUM"))

        ctr = [0]

        def psum_sc():
            ctr[0] += 1
            return pp_sc.tile([128, 1024], F32, tag="sc", name=f"sc{ctr[0]}")

        def psum_out(width=512):
            ctr[0] += 1
            return pp_out.tile([128, width], F32, tag="po", name=f"po{ctr[0]}")

        # ---- load constants ----
        wqk_sb = consts.tile([128, NKC, 512], F32R)
        wv_sb = consts.tile([128, NKC, 260], F32R)
        wp_sb = consts.tile([128, 2, 512], F32R)
        for kc in range(NKC):
            nc.sync.dma_start(out=wqk_sb[:, kc, :], in_=wqk_r[kc])
            nc.sync.dma_start(out=wv_sb[:, kc, :], in_=wv_r[kc])
        for kc in range(2):
            nc.sync.dma_start(out=wp_sb[:, kc, :], in_=wp_r[kc])
        bqk_sb = consts.tile([128, 4], F32)
        nc.sync.dma_start(out=bqk_sb, in_=bqk_d)
        bv_sb = consts.tile([1, 260], F32R)
        nc.sync.dma_start(out=bv_sb, in_=bv_d)
        bp_sb = consts.tile([128, 4], F32)
        nc.sync.dma_start(out=bp_sb, in_=bp_d)
        rs_sb = consts.tile([128, 1], F32)
        nc.sync.dma_start(out=rs_sb, in_=rs_d)
        gamma_sb = consts.tile([128, 4], F32)
        nc.sync.dma_start(out=gamma_sb, in_=gamma_d)
        beta_sb = consts.tile([128, 4], F32)
        nc.sync.dma_start(out=beta_sb, in_=beta_d)
        ones16_sb = consts.tile([128, 8], F32)
        nc.sync.dma_start(out=ones16_sb, in_=ones16_d)
        expander_sb = consts.tile([8, 128], F32)
        nc.sync.dma_start(out=expander_sb, in_=expander_d)
        onest_sb = consts.tile([1, T], F32R)
        nc.sync.dma_start(out=onest_sb, in_=onest_d)
        eps_sb = consts.tile([128, 1], F32)
        nc.vector.memset(eps_sb, EPS)
        sel_sb = consts.tile([8, 512], F32R)
        nc.sync.dma_start(out=sel_sb, in_=sel_d)

        # ---- groupnorm ----
        xs = []
        hs = []
        for kc in range(NKC):
            x_t = xpool.tile([128, T], F32, tag="x")
            for j in range(4):
                nc.sync.dma_start(
                    out=x_t[:, j * 512 : (j + 1) * 512],
                    in_=x_r[kc][:, j * 512 : (j + 1) * 512],
                )
            xs.append(x_t)

            stats = small.tile([128, 4, 6], F32, tag="stats")
            for j in range(4):
                nc.vector.bn_stats(
                    out=stats[:, j, :], in_=x_t[:, j * 512 : (j + 1) * 512]
                )
            mv = small.tile([128, 2], F32, tag="mv")
            nc.vector.bn_aggr(out=mv, in_=stats)

            # pack rhs: col0 = mean_c, col1 = E[x^2]_c = var_c + mean_c^2
            pk = small.tile([128, 2], F32, tag="pk")
            nc.vector.tensor_copy(pk[:, 0:1], mv[:, 0:1])
            nc.vector.tensor_mul(pk[:, 1:2], mv[:, 0:1], mv[:, 0:1])
            nc.vector.tensor_add(pk[:, 1:2], pk[:, 1:2], mv[:, 1:2])

            ps_g = psum_out()
            nc.tensor.matmul(
                ps_g[0:8, 0:2], ones16_sb, pk, start=True, stop=True
            )
            # gm: col0 = mean_g, col1 = rstd_g
            gsum = small.tile([8, 2], F32, tag="gsum")
            nc.vector.tensor_copy(gsum, ps_g[0:8, 0:2])
            gm = small.tile([8, 2], F32, tag="gm")
            nc.vector.tensor_copy(gm[:, 0:1], gsum[:, 0:1])
            varg = small.tile([8, 1], F32, tag="varg")
            nc.vector.tensor_mul(varg, gsum[:, 0:1], gsum[:, 0:1])
            nc.vector.tensor_sub(varg, gsum[:, 1:2], varg)
            nc.scalar.activation(varg, varg, AF.Sqrt, bias=eps_sb[0:8, :])
            nc.vector.reciprocal(gm[:, 1:2], varg)

            ps_pc = psum_out()
            nc.tensor.matmul(
                ps_pc[0:128, 0:2], expander_sb, gm, start=True, stop=True
            )
            scale = small.tile([128, 1], F32, tag="scale")
            nc.vector.tensor_mul(scale, ps_pc[:, 1:2], gamma_sb[:, kc : kc + 1])
            nbias = small.tile([128, 1], F32, tag="nbias")
            nc.vector.tensor_mul(nbias, ps_pc[:, 0:1], scale)
            nc.vector.tensor_sub(nbias, beta_sb[:, kc : kc + 1], nbias)

            h_t = big.tile([128, T], F32, tag="big")
            nc.vector.tensor_scalar(
                out=r(h_t),
                in0=x_t,
                scalar1=scale,
                scalar2=nbias,
                op0=OP.mult,
                op1=OP.add,
            )
            hs.append(h_t)

        # ---- q/k generation: m-chunks [qP0, kP0, qP1, kP1] ----
        qk_tiles = []
        for mc in range(4):
            dest = qkpool.tile([128, T], F32, tag="qk")
            qk_tiles.append(dest)
            for tc2 in range(2):
                ps = psum_sc()
                for half in range(2):
                    t0 = (tc2 * 2 + half) * 512
                    for kc in range(NKC):
                        nc.tensor.matmul(
                            ps[:, half * 512 : half * 512 + 512],
                            r(wqk_sb[:, kc, mc * 128 : mc * 128 + 128]),
                            r(hs[kc][:, t0 : t0 + 512]),
                            start=(kc == 0),
                            stop=(kc == NKC - 1),
                        )
                nc.vector.tensor_scalar(
                    out=r(dest[:, tc2 * 1024 : tc2 * 1024 + 1024]),
                    in0=ps,
                    scalar1=bqk_sb[:, mc : mc + 1],
                    scalar2=None,
                    op0=OP.add,
                )
        qpair = [qk_tiles[0], qk_tiles[2]]
        kpair = [qk_tiles[1], qk_tiles[3]]

        # ---- v generation, s-major with ones column ----
        vts = []
        for sc in range(16):
            ps = psum_sc()
            for kc in range(NKC):
                nc.tensor.matmul(
                    ps[:, 0:260],
                    r(hs[kc][:, sc * 128 : sc * 128 + 128]),
                    r(wv_sb[:, kc, :]),
                    start=(kc == 0),
                    stop=False,
                )
            nc.tensor.matmul(
                ps[:, 0:260],
                r(onest_sb[0:1, sc * 128 : sc * 128 + 128]),
                r(bv_sb),
                start=False,
                stop=True,
            )
            vt = vtpool.tile([128, 4, 65], F32, tag="vt")
            nc.vector.tensor_copy(r(vt), ps[:, 0:260])
            vts.append(vt)

        # ---- attention: heads of a pair run on PE row-group halves, both
        # heads' scoresT chunks share one [128, 1024] psum tile (col halves)
        # so the full array stays active (HAM warm) and the two QK matmuls
        # overlap.  t-chunks of 512. ----
        sums_stage = small2.tile([8, 1024], F32, tag="sums")
        att = [big.tile([128, T], F32, tag="big", name=f"att{i}") for i in range(2)]
        for pr in range(2):
            qp, kp = qpair[pr], kpair[pr]
            for tq in range(4):
                t0 = tq * 512
                outA = psum_out()
                outB = psum_out()
                for sc in range(16):
                    ps = psum_sc()
                    nc.tensor.matmul(
                        ps[:, 0:512],
                        r(kp[0:64, sc * 128 : sc * 128 + 128]),
                        r(qp[0:64, t0 : t0 + 512]),
                        start=True,
                        stop=True,
                    )
                    nc.tensor.matmul(
                        ps[:, 512:1024],
                        r(kp[64:128, sc * 128 : sc * 128 + 128]),
                        r(qp[64:128, t0 : t0 + 512]),
                        start=True,
                        stop=True,
                    )
                    pt_t = ptpool.tile([128, 1024], F32, tag="pt")
                    nc.scalar.activation(r(pt_t), ps, AF.Exp)
                    va = vts[sc][:, pr * 2 + 0, :]
                    vb = vts[sc][:, pr * 2 + 1, :]
                    nc.tensor.matmul(
                        outA[0:65, :],
                        r(va),
                        r(pt_t[:, 0:512]),
                        start=(sc == 0),
                        stop=(sc == 15),
                    )
                    nc.tensor.matmul(
                        outB[0:65, :],
                        r(vb),
                        r(pt_t[:, 512:1024]),
                        start=(sc == 0),
                        stop=(sc == 15),
                    )
                for hip, outp in ((0, outA), (1, outB)):
                    u = (pr * 2 + hip) * 2 + tq // 2
                    co = (tq % 2) * 512
                    nc.vector.tensor_copy(
                        r(att[pr][hip * 64 : hip * 64 + 64, t0 : t0 + 512]),
                        outp[0:64, :],
                    )
                    stg = ptpool.tile(
                        [128, 1024], F32, tag="pt", name=f"stg{pr}_{tq}_{hip}"
                    )
                    nc.vector.tensor_copy(stg[64:65, 0:512], outp[64:65, :])
                    nc.sync.dma_start(
                        out=sums_stage[u : u + 1, co : co + 512],
                        in_=stg[64:65, 0:512],
                    )

        # ---- softmax division + att assembly (d-major) ----
        rb = small2.tile([8, 1024], F32, tag="rb")
        scr = small2.tile([8, 1024], F32, tag="scr")
        nc.vector.reciprocal_approx_accurate(out=rb, in_=sums_stage, scratch=scr)
        rb2 = small2.tile([8, 1024], F32, tag="rb2")
        nc.vector.tensor_copy(r(rb2), rb)
        for pr in range(2):
            for hip in range(2):
                for tq in range(4):
                    u = (pr * 2 + hip) * 2 + tq // 2
                    co = (tq % 2) * 512
                    t0 = tq * 512
                    bc = psum_out()
                    nc.tensor.matmul(
                        bc[0:64, 0:512],
                        sel_sb[:, u * 64 : u * 64 + 64],
                        r(rb2[:, co : co + 512]),
                        start=True,
                        stop=True,
                    )
                    a_slc = att[pr][hip * 64 : hip * 64 + 64, t0 : t0 + 512]
                    nc.vector.tensor_mul(r(a_slc), r(a_slc), bc[0:64, :])

        # ---- projection + residual ----
        for mc in range(4):
            for tc4 in range(NTC4):
                ps = psum_sc()
                for kc2 in range(2):
                    nc.tensor.matmul(
                        ps[:, 0:512],
                        r(wp_sb[:, kc2, mc * 128 : mc * 128 + 128]),
                        r(att[kc2][:, tc4 * 512 : tc4 * 512 + 512]),
                        start=(kc2 == 0),
                        stop=(kc2 == 1),
                    )
                xz = ysb.tile([128, 512], F32, tag="y")
                nc.vector.tensor_scalar(
                    out=xz,
                    in0=xs[mc][:, tc4 * 512 : tc4 * 512 + 512],
                    scalar1=rs_sb,
                    scalar2=bp_sb[:, mc : mc + 1],
                    op0=OP.mult,
                    op1=OP.add,
                )
                yt = ysb.tile([128, 512], F32, tag="y")
                nc.vector.tensor_add(yt, ps[:, 0:512], xz)
                nc.sync.dma_start(
                    out=y_r[mc][:, tc4 * 512 : tc4 * 512 + 512], in_=yt
                )

    nc.compile()
    return nc


def _consts():
    ones16 = np.zeros((128, 8), np.float32)
    for g in range(8):
        ones16[g * 16 : (g + 1) * 16, g] = 1.0 / GS
    expander = np.zeros((8, 128), np.float32)
    for g in range(8):
        expander[g, g * 16 : (g + 1) * 16] = 1.0
    onest = np.ones((1, T), np.float32)
    sel = np.zeros((8, 512), np.float32)
    for u in range(8):
        sel[u, u * 64 : (u + 1) * 64] = 1.0
    return ones16, expander, onest, sel


def _core_weights(hg, w_qkv, b_qkv, w_proj, b_proj, gn_gamma, gn_beta):
    heads = [4 * hg + i for i in range(4)]
    qrows, krows, vrows = [], [], []
    for h in heads:
        base = h * 3 * DH
        qrows.append(np.arange(base, base + DH))
        krows.append(np.arange(base + DH, base + 2 * DH))
        vrows.append(np.arange(base + 2 * DH, base + 3 * DH))
    # m-chunks: [qP0, kP0, qP1, kP1]; each pair chunk = [head_even | head_odd]
    qk_order = np.concatenate(
        [qrows[0], qrows[1], krows[0], krows[1], qrows[2], qrows[3], krows[2], krows[3]]
    )
    wqk = round_f32r(w_qkv[qk_order].T * SCALE)
    bqk = np.ascontiguousarray((b_qkv[qk_order] * SCALE).reshape(4, 128).T)
    wv = np.zeros((C, 260), np.float32)
    bv = np.zeros((1, 260), np.float32)
    for i, vr in enumerate(vrows):
        wv[:, i * 65 : i * 65 + 64] = w_qkv[vr].T
        bv[0, i * 65 : i * 65 + 64] = b_qkv[vr]
        bv[0, i * 65 + 64] = 1.0
    wv = round_f32r(wv)
    bv = round_f32r(bv)
    att_cols = np.concatenate([np.arange(h * DH, (h + 1) * DH) for h in heads])
    wp = round_f32r(w_proj[:, att_cols].T)
    if hg == 0:
        bp = np.ascontiguousarray(b_proj.reshape(4, 128).T)
        rs = np.ones((128, 1), np.float32)
    else:
        bp = np.zeros((128, 4), np.float32)
        rs = np.zeros((128, 1), np.float32)
    gamma = np.ascontiguousarray(gn_gamma.reshape(4, 128).T)
    beta = np.ascontiguousarray(gn_beta.reshape(4, 128).T)
    return dict(
        wqk=wqk, bqk=bqk, wv=wv, bv=bv, wp=wp, bp=bp, rs=rs,
        gamma=gamma, beta=beta,
    )


def kernel(x, gn_gamma, gn_beta, w_qkv, b_qkv, w_proj, b_proj, _trace=False):
    x = np.asarray(x, np.float32)
    gn_gamma = np.asarray(gn_gamma, np.float32)
    gn_beta = np.asarray(gn_beta, np.float32)
    w_qkv = np.asarray(w_qkv, np.float32)
    b_qkv = np.asarray(b_qkv, np.float32)
    w_proj = np.asarray(w_proj, np.float32)
    b_proj = np.asarray(b_proj, np.float32)

    if "nc" not in _CACHE:
        _CACHE["nc"] = build_program()
    nc = _CACHE["nc"]

    ones16, expander, onest, sel = _consts()
    hg_consts = [
        _core_weights(hg, w_qkv, b_qkv, w_proj, b_proj, gn_gamma, gn_beta)
        for hg in range(2)
    ]
    in_maps = []
    for core in range(8):
        b, hg = core // 2, core % 2
        m = dict(hg_consts[hg])
        m["x"] = np.ascontiguousarray(x[b])
        m["ones16"] = ones16
        m["expander"] = expander
        m["onest"] = onest
        m["sel"] = sel
        in_maps.append(m)

    res = run_bass_kernel_spmd(
        nc, in_maps, core_ids=list(range(8)), trace=_trace
    )
    y = np.empty((B, C, T), np.float32)
    for b in range(B):
        y[b] = res.results[2 * b]["y"] + res.results[2 * b + 1]["y"]
    if _trace:
        _CACHE["last_results"] = res
    return y


# revision 15
# speedup vs baseline: 1.8605x; 1.1442x over previous
"""Trainium2 Bass kernel for nn_AttentionBlock (B=4, C=512, T=2048, H=8, G=32).

Sharding: 8 cores = (batch b in 0..3) x (head-group hg in 0..1, 4 heads each).
Each core computes groupnorm(x[b]) (redundantly within the pair), its heads'
q/k/v, attention, and a partial projection using its head-group's w_proj
columns.  Host sums the two partials per batch; the hg==0 core folds in the
residual x and b_proj.

On-chip layout highlights:
 - QK^T computed in scoresT[s, t] layout; two heads of a pair occupy
   partition halves 0-63 / 64-127 so their K=64 matmuls run concurrently on
   distinct PE row-groups.
 - exp via ACT psum->sbuf, no max subtraction (scores are O(5), safe in fp32).
 - V generated directly s-major (lhsT = h) and augmented with a ones column,
   so the PV matmul emits both out^T[d, t] and the softmax row sums.
 - softmax division: reciprocal of sums on DVE, broadcast across partitions
   via a K=1 ones matmul (exact fp32), then one DVE multiply.
 - big matmuls use fp32r (full-rate); tiny stat/broadcast matmuls use exact
   fp32.
"""

import sys
from contextlib import ExitStack

sys.path.insert(0, "/opt/trn_rl_repo")

import numpy as np

import concourse.bass as bass
import concourse.tile as tile
from concourse import bacc, mybir
from concourse.bass_utils import run_bass_kernel_spmd

F32 = mybir.dt.float32
F32R = mybir.dt.float32r
AF = mybir.ActivationFunctionType
OP = mybir.AluOpType

B, C, T = 4, 512, 2048
H = 8
DH = C // H          # 64
G = 32               # groupnorm groups
GS = C // G          # 16 channels per group
EPS = 1e-5
NKC = C // 128       # 4 c-chunks
NTC4 = T // 512      # 4 t-chunks of 512
SCALE = 1.0 / np.sqrt(np.sqrt(DH))

_CACHE = {}


def round_f32r(a):
    u = np.ascontiguousarray(a, np.float32).view(np.uint32)
    low = u & np.uint32(0xFFF)
    base = u & ~np.uint32(0xFFF)
    lsb = (base >> np.uint32(12)) & np.uint32(1)
    up = (low > 0x800) | ((low == 0x800) & (lsb == 1))
    out = base + (up.astype(np.uint32) << np.uint32(12))
    return out.view(np.float32)


def r(ap):
    return ap.bitcast(F32R)


def build_program():
    nc = bacc.Bacc("TRN2", target_bir_lowering=False, debug=False)

    def inp(name, shape, dt=F32):
        return nc.dram_tensor(name, shape, dt, kind="ExternalInput").ap()

    x_d = inp("x", [C, T])
    wqk_d = inp("wqk", [C, 512], F32R)
    bqk_d = inp("bqk", [128, 4])
    wv_d = inp("wv", [C, 260], F32R)
    bv_d = inp("bv", [1, 260], F32R)
    wp_d = inp("wp", [256, 512], F32R)
    bp_d = inp("bp", [128, 4])
    rs_d = inp("rs", [128, 1])
    gamma_d = inp("gamma", [128, 4])
    beta_d = inp("beta", [128, 4])
    ones16_d = inp("ones16", [128, 8])
    expander_d = inp("expander", [8, 128])
    onest_d = inp("onest", [1, T], F32R)
    sel_d = inp("sel", [4, 256], F32R)
    y_d = nc.dram_tensor("y", [C, T], F32, kind="ExternalOutput").ap()

    x_r = x_d.rearrange("(k p) t -> k p t", p=128)
    wqk_r = wqk_d.rearrange("(k p) m -> k p m", p=128)
    wv_r = wv_d.rearrange("(k p) m -> k p m", p=128)
    wp_r = wp_d.rearrange("(k p) m -> k p m", p=128)
    y_r = y_d.rearrange("(k p) t -> k p t", p=128)

    with tile.TileContext(nc) as tc, ExitStack() as ctx:
        consts = ctx.enter_context(tc.tile_pool(name="consts", bufs=1))
        xpool = ctx.enter_context(tc.tile_pool(name="xpool", bufs=4))
        big = ctx.enter_context(tc.tile_pool(name="big", bufs=4))
        qkpool = ctx.enter_context(tc.tile_pool(name="qkpool", bufs=4))
        vtpool = ctx.enter_context(tc.tile_pool(name="vtpool", bufs=16))
        ptpool = ctx.enter_context(tc.tile_pool(name="ptpool", bufs=4))
        ysb = ctx.enter_context(tc.tile_pool(name="ysb", bufs=6))
        small = ctx.enter_context(tc.tile_pool(name="small", bufs=10))
        small2 = ctx.enter_context(tc.tile_pool(name="small2", bufs=1))
        stgp = ctx.enter_context(tc.tile_pool(name="stgp", bufs=2))
        pp_sc = ctx.enter_context(tc.tile_pool(name="pp_sc", bufs=3, space="PSUM"))
        pp_out = ctx.enter_context(tc.tile_pool(name="pp_out", bufs=2, space="PSUM"))

        ctr = [0]

        def psum_sc():
            ctr[0] += 1
            return pp_sc.tile([128, 1024], F32, tag="sc", name=f"sc{ctr[0]}")

        def psum_out(width=512):
            ctr[0] += 1
            return pp_out.tile([128, width], F32, tag="po", name=f"po{ctr[0]}")

        # ---- load constants ----
        wqk_sb = consts.tile([128, NKC, 512], F32R)
        wv_sb = consts.tile([128, NKC, 260], F32R)
        wp_sb = consts.tile([128, 2, 512], F32R)
        for kc in range(NKC):
            nc.sync.dma_start(out=wqk_sb[:, kc, :], in_=wqk_r[kc])
            nc.sync.dma_start(out=wv_sb[:, kc, :], in_=wv_r[kc])
        for kc in range(2):
            nc.sync.dma_start(out=wp_sb[:, kc, :], in_=wp_r[kc])
        bqk_sb = consts.tile([128, 4], F32)
        nc.sync.dma_start(out=bqk_sb, in_=bqk_d)
        bv_sb = consts.tile([1, 260], F32R)
        nc.sync.dma_start(out=bv_sb, in_=bv_d)
        bp_sb = consts.tile([128, 4], F32)
        nc.sync.dma_start(out=bp_sb, in_=bp_d)
        rs_sb = consts.tile([128, 1], F32)
        nc.sync.dma_start(out=rs_sb, in_=rs_d)
        gamma_sb = consts.tile([128, 4], F32)
        nc.sync.dma_start(out=gamma_sb, in_=gamma_d)
        beta_sb = consts.tile([128, 4], F32)
        nc.sync.dma_start(out=beta_sb, in_=beta_d)
        ones16_sb = consts.tile([128, 8], F32)
        nc.sync.dma_start(out=ones16_sb, in_=ones16_d)
        expander_sb = consts.tile([8, 128], F32)
        nc.sync.dma_start(out=expander_sb, in_=expander_d)
        onest_sb = consts.tile([1, T], F32R)
        nc.sync.dma_start(out=onest_sb, in_=onest_d)
        eps_sb = consts.tile([128, 1], F32)
        nc.vector.memset(eps_sb, EPS)
        sel_sb = consts.tile([4, 256], F32R)
        nc.sync.dma_start(out=sel_sb, in_=sel_d)

        # ---- groupnorm ----
        xs = []
        hs = []
        for kc in range(NKC):
            x_t = xpool.tile([128, T], F32, tag="x")
            for j in range(4):
                nc.sync.dma_start(
                    out=x_t[:, j * 512 : (j + 1) * 512],
                    in_=x_r[kc][:, j * 512 : (j + 1) * 512],
                )
            xs.append(x_t)

            stats = small.tile([128, 4, 6], F32, tag="stats")
            for j in range(4):
                nc.vector.bn_stats(
                    out=stats[:, j, :], in_=x_t[:, j * 512 : (j + 1) * 512]
                )
            mv = small.tile([128, 2], F32, tag="mv")
            nc.vector.bn_aggr(out=mv, in_=stats)

            # pack rhs: col0 = mean_c, col1 = E[x^2]_c = var_c + mean_c^2
            pk = small.tile([128, 2], F32, tag="pk")
            nc.vector.tensor_copy(pk[:, 0:1], mv[:, 0:1])
            nc.vector.tensor_mul(pk[:, 1:2], mv[:, 0:1], mv[:, 0:1])
            nc.vector.tensor_add(pk[:, 1:2], pk[:, 1:2], mv[:, 1:2])

            ps_g = psum_out()
            nc.tensor.matmul(
                ps_g[0:8, 0:2], ones16_sb, pk, start=True, stop=True
            )
            # gm: col0 = mean_g, col1 = rstd_g
            gsum = small.tile([8, 2], F32, tag="gsum")
            nc.vector.tensor_copy(gsum, ps_g[0:8, 0:2])
            gm = small.tile([8, 2], F32, tag="gm")
            nc.vector.tensor_copy(gm[:, 0:1], gsum[:, 0:1])
            varg = small.tile([8, 1], F32, tag="varg")
            nc.vector.tensor_mul(varg, gsum[:, 0:1], gsum[:, 0:1])
            nc.vector.tensor_sub(varg, gsum[:, 1:2], varg)
            nc.scalar.activation(varg, varg, AF.Sqrt, bias=eps_sb[0:8, :])
            nc.vector.reciprocal(gm[:, 1:2], varg)

            ps_pc = psum_out()
            nc.tensor.matmul(
                ps_pc[0:128, 0:2], expander_sb, gm, start=True, stop=True
            )
            scale = small.tile([128, 1], F32, tag="scale")
            nc.vector.tensor_mul(scale, ps_pc[:, 1:2], gamma_sb[:, kc : kc + 1])
            nbias = small.tile([128, 1], F32, tag="nbias")
            nc.vector.tensor_mul(nbias, ps_pc[:, 0:1], scale)
            nc.vector.tensor_sub(nbias, beta_sb[:, kc : kc + 1], nbias)

            h_t = big.tile([128, T], F32, tag="big")
            nc.vector.tensor_scalar(
                out=r(h_t),
                in0=x_t,
                scalar1=scale,
                scalar2=nbias,
                op0=OP.mult,
                op1=OP.add,
            )
            hs.append(h_t)

        # ---- q/k generation: m-chunks [qP0, kP0, qP1, kP1] ----
        qk_tiles = []
        for mc in range(4):
            dest = qkpool.tile([128, T], F32, tag="qk")
            qk_tiles.append(dest)
            for tc2 in range(2):
                ps = psum_sc()
                for half in range(2):
                    t0 = (tc2 * 2 + half) * 512
                    for kc in range(NKC):
                        nc.tensor.matmul(
                            ps[:, half * 512 : half * 512 + 512],
                            r(wqk_sb[:, kc, mc * 128 : mc * 128 + 128]),
                            r(hs[kc][:, t0 : t0 + 512]),
                            start=(kc == 0),
                            stop=(kc == NKC - 1),
                        )
                nc.vector.tensor_scalar(
                    out=r(dest[:, tc2 * 1024 : tc2 * 1024 + 1024]),
                    in0=ps,
                    scalar1=bqk_sb[:, mc : mc + 1],
                    scalar2=None,
                    op0=OP.add,
                )
        qpair = [qk_tiles[0], qk_tiles[2]]
        kpair = [qk_tiles[1], qk_tiles[3]]

        # ---- v generation, s-major with ones column ----
        vts = []
        for sc in range(16):
            ps = psum_sc()
            for kc in range(NKC):
                nc.tensor.matmul(
                    ps[:, 0:260],
                    r(hs[kc][:, sc * 128 : sc * 128 + 128]),
                    r(wv_sb[:, kc, :]),
                    start=(kc == 0),
                    stop=False,
                )
            nc.tensor.matmul(
                ps[:, 0:260],
                r(onest_sb[0:1, sc * 128 : sc * 128 + 128]),
                r(bv_sb),
                start=False,
                stop=True,
            )
            vt = vtpool.tile([128, 4, 65], F32, tag="vt")
            nc.vector.tensor_copy(r(vt), ps[:, 0:260])
            vts.append(vt)

        # ---- attention: heads of a pair run on PE row-group halves, both
        # heads' scoresT chunks share one [128, 1024] psum tile (col halves)
        # so the full array stays active (HAM warm) and the two QK matmuls
        # overlap.  t-chunks of 512. ----
        sums_pair = [
            small2.tile([4, 1024], F32, tag=f"sums{i}", name=f"sums{i}")
            for i in range(2)
        ]
        att = [big.tile([128, T], F32, tag="big", name=f"att{i}") for i in range(2)]

        def divide_pair(pr):
            """softmax divide for one pair: recip of its sums, broadcast via
            K=4 selector matmul, scale att in place."""
            rbp = small2.tile([4, 1024], F32, tag="rb", name=f"rb{pr}")
            scrp = small2.tile([4, 1024], F32, tag="scr", name=f"scr{pr}")
            nc.vector.reciprocal_approx_accurate(
                out=rbp, in_=sums_pair[pr], scratch=scrp
            )
            rb2p = small2.tile([4, 1024], F32, tag="rb2", name=f"rb2{pr}")
            nc.vector.tensor_copy(r(rb2p), rbp)
            for hip in range(2):
                for tq in range(4):
                    idx = hip * 2 + tq // 2
                    co = (tq % 2) * 512
                    t0 = tq * 512
                    bc = psum_out()
                    nc.tensor.matmul(
                        bc[0:64, 0:512],
                        sel_sb[:, idx * 64 : idx * 64 + 64],
                        r(rb2p[:, co : co + 512]),
                        start=True,
                        stop=True,
                    )
                    a_slc = att[pr][hip * 64 : hip * 64 + 64, t0 : t0 + 512]
                    nc.vector.tensor_mul(r(a_slc), r(a_slc), bc[0:64, :])

        for pr in range(2):
            qp, kp = qpair[pr], kpair[pr]
            for tq in range(4):
                t0 = tq * 512
                outA = psum_out()
                outB = psum_out()
                for sc in range(16):
                    ps = psum_sc()
                    nc.tensor.matmul(
                        ps[:, 0:512],
                        r(kp[0:64, sc * 128 : sc * 128 + 128]),
                        r(qp[0:64, t0 : t0 + 512]),
                        start=True,
                        stop=True,
                    )
                    nc.tensor.matmul(
                        ps[:, 512:1024],
                        r(kp[64:128, sc * 128 : sc * 128 + 128]),
                        r(qp[64:128, t0 : t0 + 512]),
                        start=True,
                        stop=True,
                    )
                    pt_t = ptpool.tile([128, 1024], F32, tag="pt")
                    nc.scalar.activation(r(pt_t), ps, AF.Exp)
                    va = vts[sc][:, pr * 2 + 0, :]
                    vb = vts[sc][:, pr * 2 + 1, :]
                    nc.tensor.matmul(
                        outA[0:65, :],
                        r(va),
                        r(pt_t[:, 0:512]),
                        start=(sc == 0),
                        stop=(sc == 15),
                    )
                    nc.tensor.matmul(
                        outB[0:65, :],
                        r(vb),
                        r(pt_t[:, 512:1024]),
                        start=(sc == 0),
                        stop=(sc == 15),
                    )
                for hip, outp in ((0, outA), (1, outB)):
                    idx = hip * 2 + tq // 2
                    co = (tq % 2) * 512
                    nc.vector.tensor_copy(
                        r(att[pr][hip * 64 : hip * 64 + 64, t0 : t0 + 512]),
                        outp[0:64, :],
                    )
                    stg = stgp.tile(
                        [65, 512], F32, tag="stg", name=f"stg{pr}_{tq}_{hip}"
                    )
                    nc.vector.tensor_copy(stg[64:65, 0:512], outp[64:65, :])
                    nc.sync.dma_start(
                        out=sums_pair[pr][idx : idx + 1, co : co + 512],
                        in_=stg[64:65, 0:512],
                    )
            divide_pair(pr)

        # ---- projection + residual ----
        for mc in range(4):
            for tc4 in range(NTC4):
                ps = psum_sc()
                for kc2 in range(2):
                    nc.tensor.matmul(
                        ps[:, 0:512],
                        r(wp_sb[:, kc2, mc * 128 : mc * 128 + 128]),
                        r(att[kc2][:, tc4 * 512 : tc4 * 512 + 512]),
                        start=(kc2 == 0),
                        stop=(kc2 == 1),
                    )
                xz = ysb.tile([128, 512], F32, tag="y")
                nc.vector.tensor_scalar(
                    out=xz,
                    in0=xs[mc][:, tc4 * 512 : tc4 * 512 + 512],
                    scalar1=rs_sb,
                    scalar2=bp_sb[:, mc : mc + 1],
                    op0=OP.mult,
                    op1=OP.add,
                )
                yt = ysb.tile([128, 512], F32, tag="y")
                nc.vector.tensor_add(yt, ps[:, 0:512], xz)
                nc.sync.dma_start(
                    out=y_r[mc][:, tc4 * 512 : tc4 * 512 + 512], in_=yt
                )

    nc.compile()
    return nc


def _consts():
    ones16 = np.zeros((128, 8), np.float32)
    for g in range(8):
        ones16[g * 16 : (g + 1) * 16, g] = 1.0 / GS
    expander = np.zeros((8, 128), np.float32)
    for g in range(8):
        expander[g, g * 16 : (g + 1) * 16] = 1.0
    onest = np.ones((1, T), np.float32)
    sel = np.zeros((4, 256), np.float32)
    for u in range(4):
        sel[u, u * 64 : (u + 1) * 64] = 1.0
    return ones16, expander, onest, sel


def _core_weights(hg, w_qkv, b_qkv, w_proj, b_proj, gn_gamma, gn_beta):
    heads = [4 * hg + i for i in range(4)]
    qrows, krows, vrows = [], [], []
    for h in heads:
        base = h * 3 * DH
        qrows.append(np.arange(base, base + DH))
        krows.append(np.arange(base + DH, base + 2 * DH))
        vrows.append(np.arange(base + 2 * DH, base + 3 * DH))
    # m-chunks: [qP0, kP0, qP1, kP1]; each pair chunk = [head_even | head_odd]
    qk_order = np.concatenate(
        [qrows[0], qrows[1], krows[0], krows[1], qrows[2], qrows[3], krows[2], krows[3]]
    )
    wqk = round_f32r(w_qkv[qk_order].T * SCALE)
    bqk = np.ascontiguousarray((b_qkv[qk_order] * SCALE).reshape(4, 128).T)
    wv = np.zeros((C, 260), np.float32)
    bv = np.zeros((1, 260), np.float32)
    for i, vr in enumerate(vrows):
        wv[:, i * 65 : i * 65 + 64] = w_qkv[vr].T
        bv[0, i * 65 : i * 65 + 64] = b_qkv[vr]
        bv[0, i * 65 + 64] = 1.0
    wv = round_f32r(wv)
    bv = round_f32r(bv)
    att_cols = np.concatenate([np.arange(h * DH, (h + 1) * DH) for h in heads])
    wp = round_f32r(w_proj[:, att_cols].T)
    if hg == 0:
        bp = np.ascontiguousarray(b_proj.reshape(4, 128).T)
        rs = np.ones((128, 1), np.float32)
    else:
        bp = np.zeros((128, 4), np.float32)
        rs = np.zeros((128, 1), np.float32)
    gamma = np.ascontiguousarray(gn_gamma.reshape(4, 128).T)
    beta = np.ascontiguousarray(gn_beta.reshape(4, 128).T)
    return dict(
        wqk=wqk, bqk=bqk, wv=wv, bv=bv, wp=wp, bp=bp, rs=rs,
        gamma=gamma, beta=beta,
    )


def kernel(x, gn_gamma, gn_beta, w_qkv, b_qkv, w_proj, b_proj, _trace=False):
    x = np.asarray(x, np.float32)
    gn_gamma = np.asarray(gn_gamma, np.float32)
    gn_beta = np.asarray(gn_beta, np.float32)
    w_qkv = np.asarray(w_qkv, np.float32)
    b_qkv = np.asarray(b_qkv, np.float32)
    w_proj = np.asarray(w_proj, np.float32)
    b_proj = np.asarray(b_proj, np.float32)

    if "nc" not in _CACHE:
        _CACHE["nc"] = build_program()
    nc = _CACHE["nc"]

    ones16, expander, onest, sel = _consts()
    hg_consts = [
        _core_weights(hg, w_qkv, b_qkv, w_proj, b_proj, gn_gamma, gn_beta)
        for hg in range(2)
    ]
    in_maps = []
    for core in range(8):
        b, hg = core // 2, core % 2
        m = dict(hg_consts[hg])
        m["x"] = np.ascontiguousarray(x[b])
        m["ones16"] = ones16
        m["expander"] = expander
        m["onest"] = onest
        m["sel"] = sel
        in_maps.append(m)

    res = run_bass_kernel_spmd(
        nc, in_maps, core_ids=list(range(8)), trace=_trace
    )
    y = np.empty((B, C, T), np.float32)
    for b in range(B):
        y[b] = res.results[2 * b]["y"] + res.results[2 * b + 1]["y"]
    if _trace:
        _CACHE["last_results"] = res
    return y


# revision 16
# speedup vs baseline: 1.9167x; 1.0302x over previous
"""Trainium2 Bass kernel for nn_AttentionBlock (B=4, C=512, T=2048, H=8, G=32).

Sharding: 8 cores = (batch b in 0..3) x (head-group hg in 0..1, 4 heads each).
Each core computes groupnorm(x[b]) (redundantly within the pair), its heads'
q/k/v, attention, and a partial projection using its head-group's w_proj
columns.  Host sums the two partials per batch; the hg==0 core folds in the
residual x and b_proj.

On-chip layout highlights:
 - QK^T computed in scoresT[s, t] layout; two heads of a pair occupy
   partition halves 0-63 / 64-127 so their K=64 matmuls run concurrently on
   distinct PE row-groups.
 - exp via ACT psum->sbuf, no max subtraction (scores are O(5), safe in fp32).
 - V generated directly s-major (lhsT = h) and augmented with a ones column,
   so the PV matmul emits both out^T[d, t] and the softmax row sums.
 - softmax division: reciprocal of sums on DVE, broadcast across partitions
   via a K=1 ones matmul (exact fp32), then one DVE multiply.
 - big matmuls use fp32r (full-rate); tiny stat/broadcast matmuls use exact
   fp32.
"""

import sys
from contextlib import ExitStack

sys.path.insert(0, "/opt/trn_rl_repo")

import numpy as np

import concourse.bass as bass
import concourse.tile as tile
from concourse import bacc, mybir
from concourse.bass_utils import run_bass_kernel_spmd

F32 = mybir.dt.float32
F32R = mybir.dt.float32r
AF = mybir.ActivationFunctionType
OP = mybir.AluOpType

B, C, T = 4, 512, 2048
H = 8
DH = C // H          # 64
G = 32               # groupnorm groups
GS = C // G          # 16 channels per group
EPS = 1e-5
NKC = C // 128       # 4 c-chunks
NTC4 = T // 512      # 4 t-chunks of 512
SCALE = 1.0 / np.sqrt(np.sqrt(DH))

_CACHE = {}


def round_f32r(a):
    u = np.ascontiguousarray(a, np.float32).view(np.uint32)
    low = u & np.uint32(0xFFF)
    base = u & ~np.uint32(0xFFF)
    lsb = (base >> np.uint32(12)) & np.uint32(1)
    up = (low > 0x800) | ((low == 0x800) & (lsb == 1))
    out = base + (up.astype(np.uint32) << np.uint32(12))
    return out.view(np.float32)


def r(ap):
    return ap.bitcast(F32R)


def build_program():
    nc = bacc.Bacc("TRN2", target_bir_lowering=False, debug=False)

    def inp(name, shape, dt=F32):
        return nc.dram_tensor(name, shape, dt, kind="ExternalInput").ap()

    x_d = inp("x", [C, T])
    wqk_d = inp("wqk", [C, 512], F32R)
    bqk_d = inp("bqk", [128, 4])
    wv_d = inp("wv", [C, 260], F32R)
    bv_d = inp("bv", [1, 260], F32R)
    wp_d = inp("wp", [256, 512], F32R)
    bp_d = inp("bp", [128, 4])
    rs_d = inp("rs", [128, 1])
    gamma_d = inp("gamma", [128, 4])
    beta_d = inp("beta", [128, 4])
    ones16_d = inp("ones16", [128, 8])
    expander_d = inp("expander", [8, 128])
    onest_d = inp("onest", [1, T], F32R)
    sel_d = inp("sel", [4, 256], F32R)
    y_d = nc.dram_tensor("y", [C, T], F32, kind="ExternalOutput").ap()

    x_r = x_d.rearrange("(k p) t -> k p t", p=128)
    wqk_r = wqk_d.rearrange("(k p) m -> k p m", p=128)
    wv_r = wv_d.rearrange("(k p) m -> k p m", p=128)
    wp_r = wp_d.rearrange("(k p) m -> k p m", p=128)
    y_r = y_d.rearrange("(k p) t -> k p t", p=128)

    with tile.TileContext(nc) as tc, ExitStack() as ctx:
        consts = ctx.enter_context(tc.tile_pool(name="consts", bufs=1))
        xpool = ctx.enter_context(tc.tile_pool(name="xpool", bufs=4))
        big = ctx.enter_context(tc.tile_pool(name="big", bufs=4))
        qkpool = ctx.enter_context(tc.tile_pool(name="qkpool", bufs=4))
        vtpool = ctx.enter_context(tc.tile_pool(name="vtpool", bufs=16))
        ptpool = ctx.enter_context(tc.tile_pool(name="ptpool", bufs=4))
        ysb = ctx.enter_context(tc.tile_pool(name="ysb", bufs=6))
        small = ctx.enter_context(tc.tile_pool(name="small", bufs=10))
        small2 = ctx.enter_context(tc.tile_pool(name="small2", bufs=1))
        stgp = ctx.enter_context(tc.tile_pool(name="stgp", bufs=2))
        pp_sc = ctx.enter_context(tc.tile_pool(name="pp_sc", bufs=3, space="PSUM"))
        pp_out = ctx.enter_context(tc.tile_pool(name="pp_out", bufs=2, space="PSUM"))

        ctr = [0]

        def psum_sc():
            ctr[0] += 1
            return pp_sc.tile([128, 1024], F32, tag="sc", name=f"sc{ctr[0]}")

        def psum_out(width=512):
            ctr[0] += 1
            return pp_out.tile([128, width], F32, tag="po", name=f"po{ctr[0]}")

        # ---- load constants ----
        wqk_sb = consts.tile([128, NKC, 512], F32R)
        wv_sb = consts.tile([128, NKC, 260], F32R)
        wp_sb = consts.tile([128, 2, 512], F32R)
        for kc in range(NKC):
            nc.sync.dma_start(out=wqk_sb[:, kc, :], in_=wqk_r[kc])
            nc.sync.dma_start(out=wv_sb[:, kc, :], in_=wv_r[kc])
        for kc in range(2):
            nc.sync.dma_start(out=wp_sb[:, kc, :], in_=wp_r[kc])
        bqk_sb = consts.tile([128, 4], F32)
        nc.sync.dma_start(out=bqk_sb, in_=bqk_d)
        bv_sb = consts.tile([1, 260], F32R)
        nc.sync.dma_start(out=bv_sb, in_=bv_d)
        bp_sb = consts.tile([128, 4], F32)
        nc.sync.dma_start(out=bp_sb, in_=bp_d)
        rs_sb = consts.tile([128, 1], F32)
        nc.sync.dma_start(out=rs_sb, in_=rs_d)
        gamma_sb = consts.tile([128, 4], F32)
        nc.sync.dma_start(out=gamma_sb, in_=gamma_d)
        beta_sb = consts.tile([128, 4], F32)
        nc.sync.dma_start(out=beta_sb, in_=beta_d)
        ones16_sb = consts.tile([128, 8], F32)
        nc.sync.dma_start(out=ones16_sb, in_=ones16_d)
        expander_sb = consts.tile([8, 128], F32)
        nc.sync.dma_start(out=expander_sb, in_=expander_d)
        onest_sb = consts.tile([1, T], F32R)
        nc.sync.dma_start(out=onest_sb, in_=onest_d)
        eps_sb = consts.tile([128, 1], F32)
        nc.vector.memset(eps_sb, EPS)
        sel_sb = consts.tile([4, 256], F32R)
        nc.sync.dma_start(out=sel_sb, in_=sel_d)

        # ---- groupnorm ----
        xs = []
        hs = []
        for kc in range(NKC):
            x_t = xpool.tile([128, T], F32, tag="x")
            for j in range(4):
                nc.sync.dma_start(
                    out=x_t[:, j * 512 : (j + 1) * 512],
                    in_=x_r[kc][:, j * 512 : (j + 1) * 512],
                )
            xs.append(x_t)

            stats = small.tile([128, 4, 6], F32, tag="stats")
            for j in range(4):
                nc.vector.bn_stats(
                    out=stats[:, j, :], in_=x_t[:, j * 512 : (j + 1) * 512]
                )
            mv = small.tile([128, 2], F32, tag="mv")
            nc.vector.bn_aggr(out=mv, in_=stats)

            # pack rhs: col0 = mean_c, col1 = E[x^2]_c = var_c + mean_c^2
            pk = small.tile([128, 2], F32, tag="pk")
            nc.vector.tensor_copy(pk[:, 0:1], mv[:, 0:1])
            nc.vector.tensor_mul(pk[:, 1:2], mv[:, 0:1], mv[:, 0:1])
            nc.vector.tensor_add(pk[:, 1:2], pk[:, 1:2], mv[:, 1:2])

            ps_g = psum_out()
            nc.tensor.matmul(
                ps_g[0:8, 0:2], ones16_sb, pk, start=True, stop=True
            )
            # gm: col0 = mean_g, col1 = rstd_g
            gsum = small.tile([8, 2], F32, tag="gsum")
            nc.vector.tensor_copy(gsum, ps_g[0:8, 0:2])
            gm = small.tile([8, 2], F32, tag="gm")
            nc.vector.tensor_copy(gm[:, 0:1], gsum[:, 0:1])
            varg = small.tile([8, 1], F32, tag="varg")
            nc.vector.tensor_mul(varg, gsum[:, 0:1], gsum[:, 0:1])
            nc.vector.tensor_sub(varg, gsum[:, 1:2], varg)
            nc.scalar.activation(varg, varg, AF.Sqrt, bias=eps_sb[0:8, :])
            nc.vector.reciprocal(gm[:, 1:2], varg)

            ps_pc = psum_out()
            nc.tensor.matmul(
                ps_pc[0:128, 0:2], expander_sb, gm, start=True, stop=True
            )
            scale = small.tile([128, 1], F32, tag="scale")
            nc.vector.tensor_mul(scale, ps_pc[:, 1:2], gamma_sb[:, kc : kc + 1])
            nbias = small.tile([128, 1], F32, tag="nbias")
            nc.vector.tensor_mul(nbias, ps_pc[:, 0:1], scale)
            nc.vector.tensor_sub(nbias, beta_sb[:, kc : kc + 1], nbias)

            h_t = big.tile([128, T], F32, tag="big")
            nc.vector.tensor_scalar(
                out=r(h_t),
                in0=x_t,
                scalar1=scale,
                scalar2=nbias,
                op0=OP.mult,
                op1=OP.add,
            )
            hs.append(h_t)

        # ---- q/k generation: m-chunks [qP0, kP0, qP1, kP1] ----
        qk_tiles = []
        for mc in range(4):
            dest = qkpool.tile([128, T], F32, tag="qk")
            qk_tiles.append(dest)
            for tc2 in range(2):
                ps = psum_sc()
                for half in range(2):
                    t0 = (tc2 * 2 + half) * 512
                    for kc in range(NKC):
                        nc.tensor.matmul(
                            ps[:, half * 512 : half * 512 + 512],
                            r(wqk_sb[:, kc, mc * 128 : mc * 128 + 128]),
                            r(hs[kc][:, t0 : t0 + 512]),
                            start=(kc == 0),
                            stop=(kc == NKC - 1),
                        )
                nc.vector.tensor_scalar(
                    out=r(dest[:, tc2 * 1024 : tc2 * 1024 + 1024]),
                    in0=ps,
                    scalar1=bqk_sb[:, mc : mc + 1],
                    scalar2=None,
                    op0=OP.add,
                )
        qpair = [qk_tiles[0], qk_tiles[2]]
        kpair = [qk_tiles[1], qk_tiles[3]]

        # ---- v generation, s-major with ones column ----
        vts = []
        for sc in range(16):
            ps = psum_sc()
            for kc in range(NKC):
                nc.tensor.matmul(
                    ps[:, 0:260],
                    r(hs[kc][:, sc * 128 : sc * 128 + 128]),
                    r(wv_sb[:, kc, :]),
                    start=(kc == 0),
                    stop=False,
                )
            nc.tensor.matmul(
                ps[:, 0:260],
                r(onest_sb[0:1, sc * 128 : sc * 128 + 128]),
                r(bv_sb),
                start=False,
                stop=True,
            )
            vt = vtpool.tile([128, 4, 65], F32, tag="vt")
            nc.vector.tensor_copy(r(vt), ps[:, 0:260])
            vts.append(vt)

        # ---- attention: heads of a pair run on PE row-group halves, both
        # heads' scoresT chunks share one [128, 1024] psum tile (col halves)
        # so the full array stays active (HAM warm) and the two QK matmuls
        # overlap.  t-chunks of 512. ----
        sums_pair = [
            small2.tile([4, 1024], F32, tag=f"sums{i}", name=f"sums{i}")
            for i in range(2)
        ]
        att = [big.tile([128, T], F32, tag="big", name=f"att{i}") for i in range(2)]

        def divide_pair(pr, proj_tq=None):
            """softmax divide for one pair: recip of its sums, broadcast via
            K=4 selector matmul, scale att in place.  If proj_tq is given it
            is called after each t-chunk's divisions (both heads done)."""
            rbp = small2.tile([4, 1024], F32, tag="rb", name=f"rb{pr}")
            scrp = small2.tile([4, 1024], F32, tag="scr", name=f"scr{pr}")
            nc.vector.reciprocal_approx_accurate(
                out=rbp, in_=sums_pair[pr], scratch=scrp
            )
            rb2p = small2.tile([4, 1024], F32, tag="rb2", name=f"rb2{pr}")
            nc.vector.tensor_copy(r(rb2p), rbp)
            for tq in range(4):
                for hip in range(2):
                    idx = hip * 2 + tq // 2
                    co = (tq % 2) * 512
                    t0 = tq * 512
                    bc = psum_out()
                    nc.tensor.matmul(
                        bc[0:64, 0:512],
                        sel_sb[:, idx * 64 : idx * 64 + 64],
                        r(rb2p[:, co : co + 512]),
                        start=True,
                        stop=True,
                    )
                    a_slc = att[pr][hip * 64 : hip * 64 + 64, t0 : t0 + 512]
                    nc.vector.tensor_mul(r(a_slc), r(a_slc), bc[0:64, :])
                if proj_tq is not None:
                    proj_tq(tq)

        def proj_tc(tc4):
            for mc in range(4):
                ps = psum_sc()
                for kc2 in range(2):
                    nc.tensor.matmul(
                        ps[:, 0:512],
                        r(wp_sb[:, kc2, mc * 128 : mc * 128 + 128]),
                        r(att[kc2][:, tc4 * 512 : tc4 * 512 + 512]),
                        start=(kc2 == 0),
                        stop=(kc2 == 1),
                    )
                xz = ysb.tile([128, 512], F32, tag="y")
                nc.vector.tensor_scalar(
                    out=xz,
                    in0=xs[mc][:, tc4 * 512 : tc4 * 512 + 512],
                    scalar1=rs_sb,
                    scalar2=bp_sb[:, mc : mc + 1],
                    op0=OP.mult,
                    op1=OP.add,
                )
                yt = ysb.tile([128, 512], F32, tag="y")
                nc.vector.tensor_add(yt, ps[:, 0:512], xz)
                nc.sync.dma_start(
                    out=y_r[mc][:, tc4 * 512 : tc4 * 512 + 512], in_=yt
                )

        def attn_block(pr, tq):
                qp, kp = qpair[pr], kpair[pr]
                t0 = tq * 512
                outA = psum_out()
                outB = psum_out()
                for sc in range(16):
                    ps = psum_sc()
                    nc.tensor.matmul(
                        ps[:, 0:512],
                        r(kp[0:64, sc * 128 : sc * 128 + 128]),
                        r(qp[0:64, t0 : t0 + 512]),
                        start=True,
                        stop=True,
                    )
                    nc.tensor.matmul(
                        ps[:, 512:1024],
                        r(kp[64:128, sc * 128 : sc * 128 + 128]),
                        r(qp[64:128, t0 : t0 + 512]),
                        start=True,
                        stop=True,
                    )
                    pt_t = ptpool.tile([128, 1024], F32, tag="pt")
                    nc.scalar.activation(r(pt_t), ps, AF.Exp)
                    va = vts[sc][:, pr * 2 + 0, :]
                    vb = vts[sc][:, pr * 2 + 1, :]
                    nc.tensor.matmul(
                        outA[0:65, :],
                        r(va),
                        r(pt_t[:, 0:512]),
                        start=(sc == 0),
                        stop=(sc == 15),
                    )
                    nc.tensor.matmul(
                        outB[0:65, :],
                        r(vb),
                        r(pt_t[:, 512:1024]),
                        start=(sc == 0),
                        stop=(sc == 15),
                    )
                for hip, outp in ((0, outA), (1, outB)):
                    idx = hip * 2 + tq // 2
                    co = (tq % 2) * 512
                    nc.vector.tensor_copy(
                        r(att[pr][hip * 64 : hip * 64 + 64, t0 : t0 + 512]),
                        outp[0:64, :],
                    )
                    stg = stgp.tile(
                        [65, 512], F32, tag="stg", name=f"stg{pr}_{tq}_{hip}"
                    )
                    nc.vector.tensor_copy(stg[64:65, 0:512], outp[64:65, :])
                    nc.sync.dma_start(
                        out=sums_pair[pr][idx : idx + 1, co : co + 512],
                        in_=stg[64:65, 0:512],
                    )

        for tq in range(4):
            attn_block(0, tq)
        attn_block(1, 0)
        divide_pair(0)
        for tq in range(1, 4):
            attn_block(1, tq)
        divide_pair(1, proj_tq=lambda tq: proj_tc(tq))



    nc.compile()
    return nc


def _consts():
    ones16 = np.zeros((128, 8), np.float32)
    for g in range(8):
        ones16[g * 16 : (g + 1) * 16, g] = 1.0 / GS
    expander = np.zeros((8, 128), np.float32)
    for g in range(8):
        expander[g, g * 16 : (g + 1) * 16] = 1.0
    onest = np.ones((1, T), np.float32)
    sel = np.zeros((4, 256), np.float32)
    for u in range(4):
        sel[u, u * 64 : (u + 1) * 64] = 1.0
    return ones16, expander, onest, sel


def _core_weights(hg, w_qkv, b_qkv, w_proj, b_proj, gn_gamma, gn_beta):
    heads = [4 * hg + i for i in range(4)]
    qrows, krows, vrows = [], [], []
    for h in heads:
        base = h * 3 * DH
        qrows.append(np.arange(base, base + DH))
        krows.append(np.arange(base + DH, base + 2 * DH))
        vrows.append(np.arange(base + 2 * DH, base + 3 * DH))
    # m-chunks: [qP0, kP0, qP1, kP1]; each pair chunk = [head_even | head_odd]
    qk_order = np.concatenate(
        [qrows[0], qrows[1], krows[0], krows[1], qrows[2], qrows[3], krows[2], krows[3]]
    )
    wqk = round_f32r(w_qkv[qk_order].T * SCALE)
    bqk = np.ascontiguousarray((b_qkv[qk_order] * SCALE).reshape(4, 128).T)
    wv = np.zeros((C, 260), np.float32)
    bv = np.zeros((1, 260), np.float32)
    for i, vr in enumerate(vrows):
        wv[:, i * 65 : i * 65 + 64] = w_qkv[vr].T
        bv[0, i * 65 : i * 65 + 64] = b_qkv[vr]
        bv[0, i * 65 + 64] = 1.0
    wv = round_f32r(wv)
    bv = round_f32r(bv)
    att_cols = np.concatenate([np.arange(h * DH, (h + 1) * DH) for h in heads])
    wp = round_f32r(w_proj[:, att_cols].T)
    if hg == 0:
        bp = np.ascontiguousarray(b_proj.reshape(4, 128).T)
        rs = np.ones((128, 1), np.float32)
    else:
        bp = np.zeros((128, 4), np.float32)
        rs = np.zeros((128, 1), np.float32)
    gamma = np.ascontiguousarray(gn_gamma.reshape(4, 128).T)
    beta = np.ascontiguousarray(gn_beta.reshape(4, 128).T)
    return dict(
        wqk=wqk, bqk=bqk, wv=wv, bv=bv, wp=wp, bp=bp, rs=rs,
        gamma=gamma, beta=beta,
    )


def kernel(x, gn_gamma, gn_beta, w_qkv, b_qkv, w_proj, b_proj, _trace=False):
    x = np.asarray(x, np.float32)
    gn_gamma = np.asarray(gn_gamma, np.float32)
    gn_beta = np.asarray(gn_beta, np.float32)
    w_qkv = np.asarray(w_qkv, np.float32)
    b_qkv = np.asarray(b_qkv, np.float32)
    w_proj = np.asarray(w_proj, np.float32)
    b_proj = np.asarray(b_proj, np.float32)

    if "nc" not in _CACHE:
        _CACHE["nc"] = build_program()
    nc = _CACHE["nc"]

    ones16, expander, onest, sel = _consts()
    hg_consts = [
        _core_weights(hg, w_qkv, b_qkv, w_proj, b_proj, gn_gamma, gn_beta)
        for hg in range(2)
    ]
    in_maps = []
    for core in range(8):
        b, hg = core // 2, core % 2
        m = dict(hg_consts[hg])
        m["x"] = np.ascontiguousarray(x[b])
        m["ones16"] = ones16
        m["expander"] = expander
        m["onest"] = onest
        m["sel"] = sel
        in_maps.append(m)

    res = run_bass_kernel_spmd(
        nc, in_maps, core_ids=list(range(8)), trace=_trace
    )
    y = np.empty((B, C, T), np.float32)
    for b in range(B):
        y[b] = res.results[2 * b]["y"] + res.results[2 * b + 1]["y"]
    if _trace:
        _CACHE["last_results"] = res
    return y
